# revision 51
# baseline (speedup 1.0000x reference)
"""Trainium2 Bass kernel for nn_Attention_88493506167116.

Channel-attention module (per batch item b):
    F = (Wf @ X).reshape raw (N, C);  G likewise;  Hm likewise (X = x[b] as (C, N))
    S = G^T @ F  (C x C), beta = softmax(S, axis=-1)
    O = beta @ Hm^T  (C, N) -> reshape (C, W, H);  out = Wo @ O + bo

Key structure (C=256, N=4096 = 16*C): the raw reshape (C, N) -> (N, C) is a
block regrouping: F_r[16c+q, r] = Yf[c, q*C + r].  Hence with X_q = X[:, qC:(q+1)C]:
    S     = sum_q Yg_q^T Yf_q = sum_q X_q^T A X_q,   A = Wg^T Wf   (host-folded)
    out   = Wo @ O: with P = Wo @ beta, Out[o, 16c+q] = (P @ Yh_q^T)[o, c]
and Yh_q^T = YhT[qC:(q+1)C, :] where YhT = X^T Wh^T is computed directly in
transposed layout (no on-device transposes anywhere).

Sharding: pure data-parallel, batch B=8 across the 8 NeuronCores (one image
per core), zero collectives.  Host folds A = Wf.T @ Wg (lhsT layout) and
transposes weights.

Precision: the softmax is near one-hot (logit range ~500), so the S path
needs real mantissa: fp16 (10 bits) measures 4.0e-3 end-to-end (bf16's 8
bits fail at 1.8e-2 vs the 2e-2 gate).  fp16 x also halves the input DMA
stream (2MB/core) which cuts inter-core HBM contention.  The post-softmax
path (YhT evac, beta, P^T, Out matmuls) is linear and runs in bf16.

Schedule (fast path): x streams in 4 column pieces per 128-row half, with
the kc=0 pieces issued from the sync queue and kc=1 from the scalar queue
(each dynamic-DMA trigger costs ~0.7us of sequencer time, so the issues are
parallelized).  Compute is software-pipelined over piece arrival:
T(p) -> YhT(p) -> S(p) per piece, so the PE never waits on the stream.  The
last YhT quad is deferred until after the final S accumulation so it covers
the softmax latency; Out then runs in bf16 and the two output DMAs fire as
each 128-row half completes.

Hardware constraints honored: fp32/f32r matmuls self-load weights (S3_LW)
and, like DMA instructions, carry at most ONE sync wait.  Tiny fp32 warmup
matmuls cover each DMA sem lane before first use; PSUM pools are split so
each pool's tiles are only ever read by one engine (psW -> ACT, psV -> DVE).
A post-pass splits any residual multi-wait instruction into single-wait
no-ops.
"""

import numpy as np

B, C, W_DIM, H_DIM = 8, 256, 64, 64
N = W_DIM * H_DIM          # 4096
Q = N // C                 # 16
P = 128                    # partitions
NCORES = 8

_GRAPH_CACHE = {}

# x column pieces; piece p covers q-blocks PIECE_QS[p] and yht granules
# PIECE_GS[p] (granule g = spatial chunks 2g, 2g+1); granules 14,15 (the
# last yht quad) are deferred past the softmax to cover its latency
XCUTS = [0, 256, 768, 1536, 4096]
PIECE_QS = [(0, 1), (1, 3), (3, 6), (6, 16)]
PIECE_GS = [(0, 1), (1, 3), (3, 6), (6, 12)]


def _build_fast_graph():
    from contextlib import ExitStack

    import concourse.bass as bass
    import concourse.tile as tile
    from concourse import mybir

    f32 = mybir.dt.float32
    f16 = mybir.dt.float16
    bf16 = mybir.dt.bfloat16
    AF = mybir.ActivationFunctionType

    nc = bass.Bass()

    x_ext = nc.declare_dram_parameter("x", [C, N], f16, isOutput=False)
    at_ext = nc.declare_dram_parameter("atp", [P, 2 * C], f16, isOutput=False)
    wht_ext = nc.declare_dram_parameter("whtp", [P, 2 * C], f16, isOutput=False)
    wob_ext = nc.declare_dram_parameter("wobp", [P, 2 * C], bf16, isOutput=False)
    # q-major device layout: out_dev[o, q*C + c] = Out_q[o, c]; the host
    # transposes back to (c*Q + q).  This makes every Out evacuation a
    # contiguous copy and lets the output DMA pipeline at u-pair granularity.
    out_ext = nc.declare_dram_parameter("out", [C, N], bf16, isOutput=True)

    with ExitStack() as ctx:
        tc = ctx.enter_context(tile.TileContext(nc))
        cpool = ctx.enter_context(tc.tile_pool(name="cpool", bufs=1))
        psS = ctx.enter_context(tc.tile_pool(name="psS", bufs=1, space="PSUM"))
        psW = ctx.enter_context(tc.tile_pool(name="psW", bufs=3, space="PSUM"))
        psV = ctx.enter_context(tc.tile_pool(name="psV", bufs=3, space="PSUM"))

        # pool-consistent evacuation engines: psW tiles are read only by the
        # scalar engine (ACT), psV tiles only by the vector engine (DVE)
        def evacA(dst, src):
            nc.scalar.copy(dst, src)

        def evacV(dst, src):
            nc.vector.tensor_copy(dst, src)

        pick_state = [0]

        def pick():
            """alternate (pool, evac) for load balance"""
            i = pick_state[0]
            pick_state[0] += 1
            return (psV, evacV) if i % 2 == 0 else (psW, evacA)

        # ---- resident SBUF tensors -------------------------------------
        x_sb = [cpool.tile([P, N], f16, name=f"x{kc}") for kc in range(2)]
        at_sb = cpool.tile([P, 2 * C], f16, name="at")
        wht_sb = cpool.tile([P, 2 * C], f16, name="wht")
        wob_sb = cpool.tile([P, 2 * C], bf16, name="wob")

        # prime the ACT table load first thing on the scalar queue (before
        # its DMA issues and long before the first evacuation needs ACT)
        prim_src = cpool.tile([P, 1], f32, name="prim_src")
        act_prim = cpool.tile([P, 1], f32, name="act_prim")
        nc.vector.memset(prim_src[:], 1.0)
        nc.scalar.activation(act_prim[:], prim_src[:], AF.Identity,
                             bias=0.0, scale=1.0)

        # sync queue carries ALL DMAs in strict need-order (concurrent DMAs
        # share bandwidth round-robin, so the first-needed tensor must be
        # issued with nothing else in flight; a second HWDGE ring would cost
        # ~1.2us of extra queue-drain in the fixed epilogue)
        nc.sync.dma_start(out=at_sb[:], in_=at_ext[:])
        for h in range(len(XCUTS) - 1):
            for kc in range(2):
                nc.sync.dma_start(
                    out=x_sb[kc][:, XCUTS[h]:XCUTS[h + 1]],
                    in_=x_ext[kc * P:(kc + 1) * P, XCUTS[h]:XCUTS[h + 1]])
            if h == 0:
                nc.sync.dma_start(out=wht_sb[:], in_=wht_ext[:])
            elif h == 1:
                nc.sync.dma_start(out=wob_sb[:], in_=wob_ext[:])

        # ---- PE warmups ------------------------------------------------
        scratch_ps = psV.tile([P, 512], f32, name="v")

        def warmup(t):
            # native-dtype tiny matmul (fp16/bf16 tiles must not be bitcast)
            nc.tensor.matmul(scratch_ps[:, 0:1], t[:, 0:P], t[:, 0:1],
                             start=True, stop=True)

        ham_ct = cpool.tile([P, 256], f32, name="ham_ct")
        nc.vector.memset(ham_ct[:], 0.7182818)

        def dummy():
            # fp32 dummy matmul (~0.4-0.9us) keeps the PE busy so the HAM
            # p-state ramps while the first DMAs land
            nc.tensor.matmul(scratch_ps[:, 0:256], ham_ct[:, 0:P],
                             ham_ct[:], start=True, stop=True)

        dummy()
        warmup(at_sb[:, 0:P])
        dummy()
        warmup(x_sb[0][:, 0:P])
        warmup(x_sb[1][:, 0:P])

        # S accumulator PSUM tiles, pinned across the whole contraction.
        # Full-bank allocations: the two halves must not share a PSUM bank,
        # else softmax reads of S0 serialize against S1's accumulation.
        psS_full = [psS.tile([P, 2 * C], f32, name=f"S{ac}") for ac in range(2)]
        psS_t = [t[:, 0:C] for t in psS_full]

        t2_sb = [cpool.tile([P, N], f16, name=f"t2_{uc}") for uc in range(2)]
        yht_q4 = [cpool.tile([P, 4 * C], bf16, name=f"yht{u}")
                  for u in range(Q // 2)]

        s_first = [True, True]

        def emit_T(qa, nq):
            # T[:, qa*C:(qa+nq)*C] in two 128-row chunks, nq in {1, 2}
            for uc in range(2):
                pool, ev = pick()
                ps = pool.tile([P, 2 * C], f32,
                               name="v" if pool is psV else "w")
                for kc in range(2):
                    nc.tensor.matmul(
                        ps[:, 0:nq * C],
                        at_sb[:, kc * C + uc * P: kc * C + (uc + 1) * P],
                        x_sb[kc][:, qa * C:(qa + nq) * C],
                        start=(kc == 0), stop=(kc == 1),
                    )
                ev(t2_sb[uc][:, qa * C:(qa + nq) * C], ps[:, 0:nq * C])

        def emit_yht(g):
            # granule g = spatial chunks 2g, 2g+1 -> yht quad g//2, half g%2
            pool, ev = pick()
            ps = pool.tile([P, 2 * C], f32, name="v" if pool is psV else "w")
            for half in range(2):
                nch = 2 * g + half
                for kc in range(2):
                    nc.tensor.matmul(
                        ps[:, half * C:(half + 1) * C],
                        x_sb[kc][:, nch * P:(nch + 1) * P],
                        wht_sb[:, kc * C:(kc + 1) * C],
                        start=(kc == 0), stop=(kc == 1),
                    )
            ev(yht_q4[g // 2][:, (g % 2) * 2 * C:(g % 2 + 1) * 2 * C], ps[:])

        def emit_S(q, acs=(0, 1)):
            for ac in acs:
                for uc in range(2):
                    nc.tensor.matmul(
                        psS_t[ac][:],
                        x_sb[uc][:, q * C + ac * P: q * C + ac * P + P],
                        t2_sb[uc][:, q * C:(q + 1) * C],
                        start=s_first[ac] and uc == 0,
                        stop=(q == Q - 1 and uc == 1),
                        skip_group_check=True,
                    )
                s_first[ac] = False

        # ---- software pipeline over x pieces ---------------------------
        # no per-piece warmups: the first matmul touching a fresh piece
        # carries its DMA wait (the multiwait post-pass splits as needed).
        # On the last piece the two S row-halves are accumulated separately
        # so softmax of half 0 overlaps the S matmuls of half 1.
        # The softmax normalization is folded into Wo: exp writes raw bf16
        # E, and wot's contraction rows are scaled by 1/rowsum(E) instead —
        # P^T = sum_i E[i,j] (rcp_i Wo^T[i,o]).  Removes the beta casts.
        beta_sb = [cpool.tile([P, C], bf16, name=f"beta{ac}") for ac in range(2)]
        wobs_sb = cpool.tile([P, 2 * C], bf16, name="wobs")

        def emit_softmax(ac):
            negmax = cpool.tile([P, 1], f32, name=f"negmax{ac}")
            sumexp = cpool.tile([P, 1], f32, name=f"sumexp{ac}")
            rcp = cpool.tile([P, 1], f32, name=f"rcp{ac}")
            nc.vector.tensor_reduce(
                out=negmax[:], in_=psS_t[ac][:],
                axis=mybir.AxisListType.X, op=mybir.AluOpType.max, negate=True)
            nc.scalar.activation(
                beta_sb[ac][:], psS_t[ac][:], AF.Exp,
                bias=negmax[:, 0:1], scale=1.0, accum_out=sumexp[:, 0:1])
            nc.vector.reciprocal(rcp[:], sumexp[:])
            nc.vector.tensor_scalar_mul(
                wobs_sb[:, ac * C:(ac + 1) * C],
                wob_sb[:, ac * C:(ac + 1) * C], rcp[:, 0:1])

        LASTP = len(XCUTS) - 2
        for p in range(len(XCUTS) - 1):
            qa, qb = PIECE_QS[p]
            while qa < qb:
                nq = 2 if qb - qa >= 2 else 1
                emit_T(qa, nq)
                qa += nq
            for g in range(*PIECE_GS[p]):
                emit_yht(g)
            if p < LASTP:
                for q in range(*PIECE_QS[p]):
                    emit_S(q)
            else:
                for q in range(*PIECE_QS[p]):
                    emit_S(q, (0,))
                emit_softmax(0)
                for q in range(*PIECE_QS[p]):
                    emit_S(q, (1,))
                emit_softmax(1)

        # deferred yht granules cover the softmax latency; P^T is slotted
        # between them so its beta wait is also hidden
        pt_sb = [cpool.tile([P, C], bf16, name=f"pt{j}") for j in range(2)]

        def emit_pt(jpc):
            pool, ev = (psW, evacA) if jpc == 0 else (psV, evacV)
            ps = pool.tile([P, 2 * C], f32, name="w" if jpc == 0 else "v")
            for jc in range(2):
                nc.tensor.matmul(
                    ps[:, 0:C],
                    beta_sb[jc][:, jpc * P:(jpc + 1) * P],
                    wobs_sb[:, jc * C:(jc + 1) * C],
                    start=(jc == 0), stop=(jc == 1),
                )
            ev(pt_sb[jpc][:], ps[:, 0:C])

        # beta half 0's chain ran under the S half-1 matmuls, so pt0 can
        # fire immediately; the deferred granules cover beta half 1's chain
        emit_pt(0)
        emit_yht(12)
        emit_yht(13)
        emit_pt(1)
        emit_yht(14)
        emit_yht(15)

        # ================================================================
        # Out_q = P @ Yh_q^T -- bf16 matmuls (FWL), q-major PSUM output so
        # every evacuation is a contiguous copy; DMA per 4-q column group.
        # ================================================================
        for oc in range(2):
            out_sb = cpool.tile([P, Q * C], bf16, name=f"out{oc}")
            for u in range(Q // 2):
                if oc == 1 and u == Q // 2 - 1:
                    # final pair split into two single-q tiles with parallel
                    # evacuations on both engines, each followed by its own
                    # small DMA (issued from different queues), so the tail
                    # transfer is short
                    for half in range(2):
                        q = 2 * u + half
                        pool, ev = (psV, evacV) if half == 1 else (psW, evacA)
                        ps = pool.tile([P, 2 * C], f32,
                                       name="v" if pool is psV else "w")
                        for jc in range(2):
                            j = 2 * half + jc
                            nc.tensor.matmul(
                                ps[:, 0:C],
                                pt_sb[jc][:, oc * P:(oc + 1) * P],
                                yht_q4[u][:, j * C:(j + 1) * C],
                                start=(jc == 0), stop=(jc == 1),
                            )
                        ev(out_sb[:, q * C:(q + 1) * C], ps[:, 0:C])
                        if half == 0:
                            nc.sync.dma_start(
                                out=out_ext[oc * P:(oc + 1) * P,
                                            q * C:(q + 1) * C],
                                in_=out_sb[:, q * C:(q + 1) * C],
                            )
                        else:
                            # last 64KB: two 32KB partition-half DMAs (a
                            # 64KB chunk rides ONE queue at ~2.9us; halves
                            # ride two queues at ~1.4us)
                            for ph in range(2):
                                nc.sync.dma_start(
                                    out=out_ext[oc * P + ph * 64:
                                                oc * P + (ph + 1) * 64,
                                                q * C:(q + 1) * C],
                                    in_=out_sb[ph * 64:(ph + 1) * 64,
                                               q * C:(q + 1) * C],
                                )
                    continue
                pool, ev = pick()
                ps = pool.tile([P, 2 * C], f32, name="v" if pool is psV else "w")
                # rhs covers q=2u (cols 0:C) and q=2u+1 (C:2C) in one 512-wide
                # strided stream: chunks {4u+jc, 4u+2+jc} of YhT
                rhsv = yht_q4[u].rearrange("p (x y c) -> p y x c", x=2, y=2)
                for jc in range(2):
                    nc.tensor.matmul(
                        ps[:],
                        pt_sb[jc][:, oc * P:(oc + 1) * P],
                        rhsv[:, jc],
                        start=(jc == 0),
                        stop=(jc == 1),
                    )
                ev(out_sb[:, 2 * u * C:(2 * u + 2) * C], ps[:])
                if u % 2 == 1 and u < 7:
                    # quarter-row group (4 q's) complete -> fire its DMA
                    gq = u // 2
                    nc.sync.dma_start(
                        out=out_ext[oc * P:(oc + 1) * P,
                                    gq * 4 * C:(gq + 1) * 4 * C],
                        in_=out_sb[:, gq * 4 * C:(gq + 1) * 4 * C],
                    )
                elif u == 7:      # oc == 0 only: q12-15 in one DMA
                    nc.sync.dma_start(
                        out=out_ext[oc * P:(oc + 1) * P, 12 * C:],
                        in_=out_sb[:, 12 * C:],
                    )
                elif oc == 1 and u == 6:
                    # q12-13 rides ahead of the split final pair
                    nc.sync.dma_start(
                        out=out_ext[oc * P:(oc + 1) * P, 12 * C:14 * C],
                        in_=out_sb[:, 12 * C:14 * C],
                    )

    return nc


def _build_bias_graph():
    """General path with biases: full fp32, materialized Yf/Yg."""
    from contextlib import ExitStack

    import concourse.bass as bass
    import concourse.tile as tile
    from concourse import mybir

    f32 = mybir.dt.float32
    AF = mybir.ActivationFunctionType

    nc = bass.Bass()

    NW = 8
    x_ext = nc.declare_dram_parameter("x", [C, N], f32, isOutput=False)
    wpk_ext = nc.declare_dram_parameter("wpk", [P, NW * C], f32, isOutput=False)
    bpk_ext = nc.declare_dram_parameter("bpk", [P, 6], f32, isOutput=False)
    bhw_ext = nc.declare_dram_parameter("bhw", [1, 3 * C], f32, isOutput=False)
    out_ext = nc.declare_dram_parameter("out", [C, N], f32, isOutput=True)

    with ExitStack() as ctx:
        tc = ctx.enter_context(tile.TileContext(nc))
        cpool = ctx.enter_context(tc.tile_pool(name="cpool", bufs=1))
        psS = ctx.enter_context(tc.tile_pool(name="psS", bufs=1, space="PSUM"))
        psW = ctx.enter_context(tc.tile_pool(name="psW", bufs=3, space="PSUM"))

        def evacA(dst, src):
            nc.scalar.copy(dst, src)

        x_sb = [cpool.tile([P, N], f32, name=f"x{kc}") for kc in range(2)]
        wpk_sb = cpool.tile([P, NW, C], f32, name="wpk")
        nc.sync.dma_start(out=wpk_sb.rearrange("p a b -> p (a b)"), in_=wpk_ext[:])
        BXCUTS = [0, 512, 1536, 2560, N]
        for h in range(len(BXCUTS) - 1):
            for kc in range(2):
                nc.sync.dma_start(
                    out=x_sb[kc][:, BXCUTS[h]:BXCUTS[h + 1]],
                    in_=x_ext[kc * P:(kc + 1) * P, BXCUTS[h]:BXCUTS[h + 1]])

        wft_sb = [wpk_sb[:, 0 + kc, :] for kc in range(2)]
        wgt_sb = [wpk_sb[:, 2 + kc, :] for kc in range(2)]
        wht_sb = [wpk_sb[:, 4 + kc, :] for kc in range(2)]
        wot_sb = [wpk_sb[:, 6 + kc, :] for kc in range(2)]
        bpk_sb = cpool.tile([P, 6], f32, name="bpk")
        bhw_sb = cpool.tile([1, 3 * C], f32, name="bhw")
        nc.sync.dma_start(out=bpk_sb[:], in_=bpk_ext[:])
        nc.sync.dma_start(out=bhw_sb[:], in_=bhw_ext[:])
        bf_sb = [bpk_sb[:, 0 + kc:1 + kc] for kc in range(2)]
        bg_sb = [bpk_sb[:, 2 + kc:3 + kc] for kc in range(2)]
        bo_sb = [bpk_sb[:, 4 + kc:5 + kc] for kc in range(2)]
        bh2_row = bhw_sb[0:1, 0:2 * C]       # [bh | bh]
        wosum_row = bhw_sb[0:1, 2 * C:3 * C]

        scratch_ps = psW.tile([P, 512], f32, name="w")

        def warmup(t):
            # native-dtype tiny matmul (fp16/bf16 tiles must not be bitcast)
            nc.tensor.matmul(scratch_ps[:, 0:1], t[:, 0:P], t[:, 0:1],
                             start=True, stop=True)

        warmup(wpk_sb[:, 0, 0:P])
        warmup(x_sb[0][:, 0:P])
        warmup(x_sb[1][:, 0:P])
        for h in range(1, 4):
            warmup(x_sb[0][:, BXCUTS[h]:BXCUTS[h] + P])
            warmup(x_sb[1][:, BXCUTS[h]:BXCUTS[h] + P])
        nc.tensor.matmul(scratch_ps[0:1, 0:1], bhw_sb[0:1, 0:1],
                         bhw_sb[0:1, 0:1], start=True, stop=True)
        act_scr = cpool.tile([P, 1], f32, name="act_scr")
        nc.scalar.copy(act_scr[:], bpk_sb[:, 0:1])

        psS_t = [psS.tile([P, C], f32, name=f"S{ac}") for ac in range(2)]

        yf_sb = [cpool.tile([P, N], f32, name=f"yf{mc}") for mc in range(2)]
        yg_sb = [cpool.tile([P, N], f32, name=f"yg{mc}") for mc in range(2)]
        for mc in range(2):
            for nb in range(8):
                nsl = slice(nb * 512, (nb + 1) * 512)
                ps = psW.tile([P, 512], f32, name="w")
                for kc in range(2):
                    nc.tensor.matmul(
                        ps[:], wft_sb[kc][:, mc * P:(mc + 1) * P],
                        x_sb[kc][:, nsl], start=(kc == 0), stop=(kc == 1))
                nc.scalar.activation(yf_sb[mc][:, nsl], ps[:], AF.Identity,
                                     bias=bf_sb[mc], scale=1.0)
                ps = psW.tile([P, 512], f32, name="w")
                for kc in range(2):
                    nc.tensor.matmul(
                        ps[:], wgt_sb[kc][:, mc * P:(mc + 1) * P],
                        x_sb[kc][:, nsl], start=(kc == 0), stop=(kc == 1))
                nc.scalar.activation(yg_sb[mc][:, nsl], ps[:], AF.Identity,
                                     bias=bg_sb[mc], scale=1.0)
        for ac in range(2):
            for q in range(Q):
                for kc in range(2):
                    nc.tensor.matmul(
                        psS_t[ac][:],
                        yg_sb[kc][:, q * C + ac * P: q * C + ac * P + P],
                        yf_sb[kc][:, q * C:(q + 1) * C],
                        start=(q == 0 and kc == 0),
                        stop=(q == Q - 1 and kc == 1),
                    )

        yht_q4 = [cpool.tile([P, 4 * C], f32, name=f"yht{u}")
                  for u in range(Q // 2)]
        for u in range(Q // 2):
            for g in range(2):
                ps = psW.tile([P, 2 * C], f32, name="w")
                for half in range(2):
                    nch = 4 * u + 2 * g + half
                    for kc in range(2):
                        nc.tensor.matmul(
                            ps[:, half * C:(half + 1) * C],
                            x_sb[kc][:, nch * P:(nch + 1) * P],
                            wht_sb[kc][:],
                            start=(kc == 0), stop=(kc == 1),
                        )
                evacA(yht_q4[u][:, g * 2 * C:(g + 1) * 2 * C], ps[:])

        beta_sb = [cpool.tile([P, C], f32, name=f"beta{ac}") for ac in range(2)]
        for ac in range(2):
            negmax = cpool.tile([P, 1], f32, name=f"negmax{ac}")
            sumexp = cpool.tile([P, 1], f32, name=f"sumexp{ac}")
            rcp = cpool.tile([P, 1], f32, name=f"rcp{ac}")
            expo = cpool.tile([P, C], f32, name=f"expo{ac}")
            nc.vector.tensor_reduce(
                out=negmax[:], in_=psS_t[ac][:],
                axis=mybir.AxisListType.X, op=mybir.AluOpType.max, negate=True)
            nc.scalar.activation(
                expo[:], psS_t[ac][:], AF.Exp,
                bias=negmax[:, 0:1], scale=1.0, accum_out=sumexp[:, 0:1])
            nc.vector.reciprocal(rcp[:], sumexp[:])
            nc.scalar.activation(beta_sb[ac][:], expo[:], AF.Copy,
                                 bias=0.0, scale=rcp[:, 0:1])

        pt_sb = [cpool.tile([P, C], f32, name=f"pt{j}") for j in range(2)]
        for jpc in range(2):
            ps = psW.tile([P, 2 * C], f32, name="w")
            for jc in range(2):
                nc.tensor.matmul(
                    ps[:, 0:C],
                    beta_sb[jc][:, jpc * P:(jpc + 1) * P],
                    wot_sb[jc][:],
                    start=(jc == 0), stop=(jc == 1),
                )
            evacA(pt_sb[jpc][:], ps[:, 0:C])

        for oc in range(2):
            out_sb = cpool.tile([P, C, Q], f32, name=f"out{oc}")
            for u in range(Q // 2):
                ps = psW.tile([P, 2 * C], f32, name="w")
                rhsv = yht_q4[u].rearrange("p (x y c) -> p y x c", x=2, y=2)
                for jc in range(2):
                    nc.tensor.matmul(
                        ps[:],
                        pt_sb[jc][:, oc * P:(oc + 1) * P],
                        rhsv[:, jc],
                        start=(jc == 0),
                        stop=False,
                    )
                nc.tensor.matmul(
                    ps[:],
                    wosum_row[:, oc * P:(oc + 1) * P],
                    bh2_row[:],
                    start=False, stop=True,
                )
                nc.scalar.activation(
                    out_sb[:, :, 2 * u:2 * u + 2],
                    ps.rearrange("p (h c) -> p c h", h=2),
                    AF.Identity, bias=bo_sb[oc], scale=1.0)
            nc.sync.dma_start(
                out=out_ext[oc * P:(oc + 1) * P, :],
                in_=out_sb.rearrange("p c q -> p (c q)"),
            )

    return nc


def _split_multiwait_insts(nc, max_waits: int = 1):
    """walrus rejects instructions carrying more than one sync wait; hoist
    extra waits onto same-engine no-ops placed immediately before."""
    from concourse import mybir

    nop_id = 0
    for fn in nc.m.functions:
        for blk in fn.blocks:
            insts = list(blk.instructions)
            new_list = []
            changed = False
            for inst in insts:
                si = inst.sync_info
                if si is not None and len(si.on_wait) > max_waits:
                    waits = list(si.on_wait)
                    for w in waits[:-max_waits]:
                        nop = mybir.InstNoOp(name=f"I-waitnop{nop_id}", ins=[],
                                             outs=[])
                        nop_id += 1
                        nop.engine = inst.engine
                        nop.sync_info = mybir.SyncInfo(on_wait=[w], on_update=[])
                        new_list.append(nop)
                    inst.sync_info = mybir.SyncInfo(
                        on_wait=waits[-max_waits:],
                        on_update=list(si.on_update),
                    )
                    changed = True
                new_list.append(inst)
            if changed:
                blk.instructions = new_list
    return nc


def _get_graph(use_bias: bool):
    key = bool(use_bias)
    if key not in _GRAPH_CACHE:
        builder = _build_bias_graph if key else _build_fast_graph
        _GRAPH_CACHE[key] = _split_multiwait_insts(builder())
    return _GRAPH_CACHE[key]


def _make_in_maps(inputs, use_bias):
    import ml_dtypes

    x = np.ascontiguousarray(np.asarray(inputs["x"], dtype=np.float32))
    Wf = np.asarray(inputs["Wf"], dtype=np.float32)
    Wg = np.asarray(inputs["Wg"], dtype=np.float32)
    Wh = np.asarray(inputs["Wh"], dtype=np.float32)
    Wo = np.asarray(inputs["Wo"], dtype=np.float32)

    wht = np.ascontiguousarray(Wh.T)
    wot = np.ascontiguousarray(Wo.T)

    def chunks2(w):
        # [2C] columns = [rows 0:128 | rows 128:256] of w, partition-major
        return np.ascontiguousarray(
            np.concatenate([w[:P], w[P:]], axis=1).reshape(P, 2 * C))

    if use_bias:
        def swizzle(wlist):
            chunks = []
            for w in wlist:
                chunks.append(w[:P])
                chunks.append(w[P:])
            arr = np.stack(chunks, axis=0)           # (NW, P, C)
            return np.ascontiguousarray(
                arr.transpose(1, 0, 2).reshape(P, -1))

        bf = np.asarray(inputs["bf"], np.float32)
        bg = np.asarray(inputs["bg"], np.float32)
        bh = np.asarray(inputs["bh"], np.float32)
        bo = np.asarray(inputs["bo"], np.float32)
        wpk = swizzle([Wf.T, Wg.T, wht, wot])
        bpk = np.stack([bf[:P], bf[P:], bg[:P], bg[P:], bo[:P], bo[P:]], axis=1)
        bhw = np.concatenate([bh, bh, Wo.sum(axis=1)]).reshape(1, 3 * C)
        common = {
            "wpk": wpk,
            "bpk": np.ascontiguousarray(bpk),
            "bhw": np.ascontiguousarray(bhw),
        }
    else:
        at = Wf.T @ Wg
        common = {
            "atp": chunks2(at).astype(np.float16),
            "whtp": chunks2(wht).astype(np.float16),
            "wobp": chunks2(wot).astype(ml_dtypes.bfloat16),
        }
        return [
            {"x": np.ascontiguousarray(x[i].reshape(C, N).astype(np.float16)),
             **common}
            for i in range(NCORES)
        ]

    return [
        {"x": np.ascontiguousarray(x[i].reshape(C, N)), **common}
        for i in range(NCORES)
    ]


def kernel(x, Wf, bf, Wg, bg, Wh, bh, Wo, bo):
    from concourse.bass_utils import run_bass_kernel_spmd

    inputs = {"x": x, "Wf": Wf, "bf": bf, "Wg": Wg, "bg": bg,
              "Wh": Wh, "bh": bh, "Wo": Wo, "bo": bo}
    use_bias = bool(
        np.any(np.asarray(bf)) or np.any(np.asarray(bg))
        or np.any(np.asarray(bh)) or np.any(np.asarray(bo))
    )
    nc = _get_graph(use_bias)
    in_maps = _make_in_maps(inputs, use_bias)
    out = None
    last_err = None
    for attempt in range(3):
        try:
            res = run_bass_kernel_spmd(nc, in_maps, list(range(NCORES)))
            # materialize INSIDE the retry: execution errors surface lazily
            # when the jax result arrays are converted to numpy
            out = np.stack(
                [np.asarray(res.results[i]["out"]) for i in range(NCORES)])
            break
        except Exception as e:  # transient device wedge (NRT unrecoverable)
            last_err = e
            if "UNRECOVERABLE" not in str(e) and "UNAVAILABLE" not in str(e):
                raise
            import time
            time.sleep(10)
    if out is None:
        raise last_err
    out = out.astype(np.float32)
    if not use_bias:
        # device emitted q-major columns (q*C + c); reorder to (c*Q + q)
        out = out.reshape(B, C, Q, C).transpose(0, 1, 3, 2)
    return np.ascontiguousarray(out.reshape(B, C, W_DIM, H_DIM))
